# revision 24
# baseline (speedup 1.0000x reference)
"""Trainium2 Bass kernel for DiffPool forward (nn_DiffPool_56573309223698).

Algorithm (8-core SPMD, one NEFF):
  stage B/C (per core, its 2048-node shard, 16 chunks of 128 nodes):
    logits = x @ Wa (+ba)  -> softmax (exp on ScalarE with fused row-sum,
    reciprocal+scale on VectorE) -> S shard (f32 output), S bf16 copy,
    gather-table copy (GTYPE) -> local DRAM
    h = x @ Wf (+bf);  pooled_x partial += S_chunk^T @ h_chunk  (f32, PSUM)
  AllGather of the GTYPE S table (each core ends with the full [16384,512]
    table in its HBM).
  stage E (sparse pooled adjacency, edges sorted by 128-row block):
    per block: dma_gather of S[col] rows for that block's edges (GTYPE),
    one-hot matmuls  U_block = sum_chunks onehot(w)^T @ Gc_chunk  (PSUM f32),
    U -> bf16, P += S_bf16_block^T @ U_block (PSUM f32, 4 banks)
  Host: sums the per-core P / pooled_x partials, pooled_adj = P + P^T.

The one-hot tiles (lhsT of the scatter matmuls) and the sorted/deduped
(set-semantics, last-write-wins) edge layout are prepared host-side as part
of sharding; all FLOPs run on device.
"""

import os
import sys

import numpy as np
import ml_dtypes

if "/opt/trn_rl_repo" not in sys.path:
    sys.path.insert(0, "/opt/trn_rl_repo")

N = 16384
CIN = 128
K = 512
NCORES = 8
SHARD = N // NCORES          # 2048 nodes per core
BLKS = SHARD // 128          # 16 row-blocks per core
NBLK = N // 128              # 128 global row-blocks

_GD = os.environ.get("KERNEL_GATHER_DTYPE", "fp8")
if _GD == "fp8":
    G_NP = ml_dtypes.float8_e4m3
    SCALE = 256.0
else:
    G_NP = ml_dtypes.bfloat16
    SCALE = 1.0

_cache = {}


def _build(CHS, ba_nonzero, bf_nonzero, cost_only=False, reps=1, no_cc=False, skip_gather=False):
    CH = sum(CHS)
    import concourse.bacc as bacc
    import concourse.mybir as mybir
    import concourse.tile as tile

    G_MB = mybir.dt.float8e4 if _GD == "fp8" else mybir.dt.bfloat16
    f32 = mybir.dt.float32
    bf16 = mybir.dt.bfloat16
    i16 = mybir.dt.int16
    AF = mybir.ActivationFunctionType

    NQ = int(os.environ.get("KERNEL_NQ", "4"))
    nc = bacc.Bacc("TRN2", target_bir_lowering=False, debug=False,
                   num_devices=1 if cost_only else NCORES,
                   num_swdge_queues=NQ)

    xT = nc.dram_tensor("xT", [CIN, SHARD], f32, kind="ExternalInput")
    wa = nc.dram_tensor("wa", [CIN, K], f32, kind="ExternalInput")
    wf = nc.dram_tensor("wf", [CIN, CIN], f32, kind="ExternalInput")
    oh = nc.dram_tensor("oh", [BLKS, 128, CH * 128], G_MB, kind="ExternalInput")
    idxs = nc.dram_tensor("idxs", [128, BLKS * CH * 8], i16, kind="ExternalInput")
    ba_bc = nc.dram_tensor("ba_bc", [128, K], f32,
                           kind="ExternalInput") if ba_nonzero else None
    bf_bc = nc.dram_tensor("bf_bc", [128, CIN], f32,
                           kind="ExternalInput") if bf_nonzero else None

    s_out = nc.dram_tensor("s_out", [SHARD, K], f32, kind="ExternalOutput")
    p_out = nc.dram_tensor("p_out", [K, K], f32, kind="ExternalOutput")
    px_out = nc.dram_tensor("px_out", [K, CIN], f32, kind="ExternalOutput")

    tab_local = nc.dram_tensor("tab_local", [SHARD, K], G_MB)
    asp = "Local" if cost_only else "Shared"
    NSPLIT = len(CHS)
    tab_half = [nc.dram_tensor(f"tab_all{g}", [N // NSPLIT, K], G_MB,
                               addr_space=asp) for g in range(NSPLIT)]

    with tile.TileContext(nc) as tc:
        with (
            tc.tile_pool(name="const", bufs=1) as const,
            tc.tile_pool(name="work", bufs=3) as work,
        ):
            xt_sb = const.tile([CIN, SHARD], f32)
            nc.sync.dma_start(xt_sb[:], xT[:])
            wa_sb = const.tile([CIN, K], f32)
            nc.sync.dma_start(wa_sb[:], wa[:])
            wf_sb = const.tile([CIN, CIN], f32)
            nc.sync.dma_start(wf_sb[:], wf[:])
            idx_sb = const.tile([128, BLKS * CH * 8], i16)
            nc.sync.dma_start(idx_sb[:], idxs[:])
            ba_sb = bf_sb = None
            if ba_nonzero:
                ba_sb = const.tile([128, K], f32)
                nc.sync.dma_start(ba_sb[:], ba_bc[:])
            if bf_nonzero:
                bf_sb = const.tile([128, CIN], f32)
                nc.sync.dma_start(bf_sb[:], bf_bc[:])

            s_bf = const.tile([128, BLKS, K], bf16)       # S shard, bf16

            for rep in range(reps):
                # ---------------- stage B/C ----------------
                with (
                    tc.tile_pool(name=f"ps_l{rep}", bufs=2, space="PSUM") as ps_l,
                    tc.tile_pool(name=f"ps_h{rep}", bufs=2, space="PSUM") as ps_h,
                    tc.tile_pool(name=f"ps_px{rep}", bufs=1, space="PSUM") as ps_px,
                ):
                    px_ps = [ps_px.tile([128, CIN], f32,
                                        name=f"px_ps{rep}_{kb}", tag=f"px{kb}")
                             for kb in range(4)]
                    for n in range(BLKS):
                        l_ps = ps_l.tile([128, K], f32)
                        nc.tensor.matmul(l_ps[:], xt_sb[:, n * 128:(n + 1) * 128],
                                         wa_sb[:], start=True, stop=True)
                        e_sb = work.tile([128, K], f32, tag="e_sb")
                        ssum = work.tile([128, 1], f32, tag="ssum")
                        if ba_nonzero:
                            l2 = work.tile([128, K], f32, tag="l2")
                            nc.vector.tensor_add(l2[:], l_ps[:], ba_sb[:])
                            nc.scalar.activation(e_sb[:], l2[:], AF.Exp,
                                                 accum_out=ssum[:])
                        else:
                            nc.scalar.activation(e_sb[:], l_ps[:], AF.Exp,
                                                 accum_out=ssum[:])
                        rec = work.tile([128, 1], f32, tag="rec")
                        nc.vector.reciprocal(rec[:], ssum[:])
                        s_f32 = work.tile([128, K], f32, tag="s_f32")
                        nc.vector.tensor_scalar_mul(s_f32[:], e_sb[:], rec[:])
                        nc.sync.dma_start(s_out[n * 128:(n + 1) * 128, :], s_f32[:])
                        nc.vector.tensor_copy(s_bf[:, n, :], s_f32[:])
                        if _GD == "fp8":
                            s_g = work.tile([128, K], G_MB, tag="s_g")
                            nc.scalar.activation(s_g[:], s_f32[:], AF.Copy,
                                                 scale=SCALE)
                            nc.sync.dma_start(tab_local[n * 128:(n + 1) * 128, :],
                                              s_g[:])
                        else:
                            nc.sync.dma_start(tab_local[n * 128:(n + 1) * 128, :],
                                              s_bf[:, n, :])

                        h_ps = ps_h.tile([128, CIN], f32)
                        nc.tensor.matmul(h_ps[:], xt_sb[:, n * 128:(n + 1) * 128],
                                         wf_sb[:], start=True, stop=True)
                        h_sb = work.tile([128, CIN], f32, tag="h_sb")
                        if bf_nonzero:
                            nc.vector.tensor_add(h_sb[:], h_ps[:], bf_sb[:])
                        else:
                            nc.scalar.copy(h_sb[:], h_ps[:])
                        for kb in range(4):
                            nc.tensor.matmul(px_ps[kb][:],
                                             s_f32[:, kb * 128:(kb + 1) * 128],
                                             h_sb[:],
                                             start=(n == 0), stop=(n == BLKS - 1))

                    px_sb = work.tile([128, 4, CIN], f32, tag="px_sb")
                    for kb in range(4):
                        nc.vector.tensor_copy(px_sb[:, kb, :], px_ps[kb][:])
                    nc.sync.dma_start(
                        px_out[:].rearrange("(a p) c -> p a c", p=128), px_sb[:])

                # ---------------- all-gather (two halves) ----------------
                if not (cost_only or no_cc):
                    for g in range(NSPLIT):
                        nc.gpsimd.collective_compute(
                            "AllGather", mybir.AluOpType.bypass,
                            replica_groups=[list(range(NCORES))],
                            ins=[tab_local[g * SHARD // NSPLIT:
                                           (g + 1) * SHARD // NSPLIT, :].opt()],
                            outs=[tab_half[g][:].opt()],
                        )

                # ---------------- stage E ----------------
                with (
                    tc.tile_pool(name=f"ps_u{rep}", bufs=2, space="PSUM") as ps_u,
                    tc.tile_pool(name=f"ps_p{rep}", bufs=1, space="PSUM") as ps_p,
                    tc.tile_pool(name=f"ohp{rep}", bufs=int(os.environ.get("KERNEL_OHBUFS", "4"))) as ohp,
                    tc.tile_pool(name=f"gcp{rep}", bufs=int(os.environ.get("KERNEL_GCBUFS", "7"))) as gcp,
                ):
                    p_ps = [ps_p.tile([128, K], f32,
                                      name=f"p_ps{rep}_{kb}", tag=f"p{kb}")
                            for kb in range(4)]
                    for b in range(BLKS):
                        oh_sb = ohp.tile([128, CH * 128], G_MB)
                        nc.sync.dma_start(oh_sb[:], oh[b])
                        gc = gcp.tile([128, CH, K], G_MB)
                        if skip_gather:
                            nc.vector.memset(gc[:, 0, 0:16], 0.0)
                        else:
                            c0 = 0
                            for g, chs in enumerate(CHS):
                                nc.gpsimd.dma_gather(
                                    gc[:, c0:c0 + chs, :], tab_half[g][:],
                                    idx_sb[:, b * CH * 8 + c0 * 8:
                                           b * CH * 8 + (c0 + chs) * 8],
                                    chs * 128, chs * 128, K,
                                    single_packet=False,
                                    queue_num=(b * NSPLIT + g) % NQ)
                                c0 += chs
                        u_ps = ps_u.tile([128, K], f32)
                        use_dr = (_GD == "fp8"
                                  and int(os.environ.get("KERNEL_DR", "1")))
                        if use_dr:
                            ohv = oh_sb[:].rearrange("p (c t r) -> p c t r",
                                                     t=2, r=128)
                            for ch in range(CH // 2):
                                nc.tensor.matmul(
                                    u_ps[:], ohv[:, ch, :, :],
                                    gc[:, 2 * ch:2 * ch + 2, :],
                                    start=(ch == 0), stop=(ch == CH // 2 - 1),
                                    perf_mode=mybir.MatmulPerfMode.DoubleRow)
                        else:
                            for ch in range(CH):
                                nc.tensor.matmul(u_ps[:],
                                                 oh_sb[:, ch * 128:(ch + 1) * 128],
                                                 gc[:, ch, :],
                                                 start=(ch == 0),
                                                 stop=(ch == CH - 1))
                        u_bf = work.tile([128, K], bf16, tag="u_bf")
                        nc.vector.tensor_copy(u_bf[:], u_ps[:])
                        for kb in range(4):
                            nc.tensor.matmul(p_ps[kb][:],
                                             s_bf[:, b, kb * 128:(kb + 1) * 128],
                                             u_bf[:],
                                             start=(b == 0), stop=(b == BLKS - 1))

                    for kb in range(4):
                        p_sb = work.tile([128, K], f32, tag="p_sb")
                        nc.vector.tensor_copy(p_sb[:], p_ps[kb][:])
                        nc.sync.dma_start(p_out[kb * 128:(kb + 1) * 128, :],
                                          p_sb[:])

    nc.compile()
    return nc


def _prep(inputs):
    """Host-side sharding prep: dedup (set semantics), sort by row block,
    pad to 128-edge chunks, build one-hot tiles + wrapped gather indices."""
    x = np.ascontiguousarray(np.asarray(inputs["x"], dtype=np.float32))
    ei = np.asarray(inputs["edge_index"])
    ew = np.asarray(inputs["edge_weight"], dtype=np.float32)
    Wa = np.ascontiguousarray(np.asarray(inputs["Wa"], dtype=np.float32))
    ba = np.asarray(inputs["ba"], dtype=np.float32)
    Wf = np.ascontiguousarray(np.asarray(inputs["Wf"], dtype=np.float32))
    bf = np.asarray(inputs["bf"], dtype=np.float32)

    r = ei[0].astype(np.int64)
    c = ei[1].astype(np.int64)
    lin = r * N + c
    order = np.argsort(lin, kind="stable")
    lin_s = lin[order]
    keep = np.ones(len(order), bool)
    keep[:-1] = lin_s[:-1] != lin_s[1:]      # keep last write of duplicates
    order = order[keep]
    r_s = r[order].astype(np.int32)
    c_s = c[order].astype(np.int32)
    w_s = ew[order]

    blk = r_s >> 7
    # The table is all-gathered in two halves: half g holds, for each rank m,
    # that rank's shard chunks [g*8, (g+1)*8). Global node u decomposes as
    # rank m = u >> 11, chunk c = (u >> 7) & 15, p = u & 127; its row in its
    # half-table is m*(SHARD//2) + (c - g*8)*128 + p with g = c // 8.
    NSPLIT = int(os.environ.get("KERNEL_TSPLIT", "1"))
    chunk_in_shard = (c_s >> 7) & (BLKS - 1)
    half = (chunk_in_shard // (BLKS // NSPLIT)).astype(np.int64)
    rank = (c_s >> 11).astype(np.int64)
    tabrow = (rank * (SHARD // NSPLIT)
              + (chunk_in_shard - half * (BLKS // NSPLIT)) * 128 + (c_s & 127))
    # sort by (blk, half, col) so each (blk, half) slot group is contiguous
    bkey = (blk.astype(np.int64) * NSPLIT + half) * N + c_s
    order2 = np.argsort(bkey, kind="stable")
    r_s, c_s, w_s, blk = r_s[order2], c_s[order2], w_s[order2], blk[order2]
    half, tabrow, bkey = half[order2], tabrow[order2], bkey[order2]
    # slot per unique (blk, half, col); edges sharing one share a gather slot
    # (their lhsT column just has several nonzeros - still a valid matmul)
    newslot = np.ones(len(bkey), bool)
    newslot[1:] = bkey[1:] != bkey[:-1]
    grp = blk * NSPLIT + half
    counts2 = np.bincount(grp[newslot], minlength=NBLK * NSPLIT)
    CHS = []
    for g in range(NSPLIT):
        chg = max(1, int(-(-counts2[g::NSPLIT].max() // 128)))
        CHS.append(chg + chg % 2)                       # even for DoubleRow
    CH = sum(CHS)
    slots = CH * 128

    # slot rank within the block: half-0 slots first (capacity CHS[0]*128)
    slot_glob = np.cumsum(newslot) - 1
    sl_grp = grp[newslot]
    sl_ids = slot_glob[newslot]
    sl_first = np.ones(len(sl_grp), bool)
    sl_first[1:] = sl_grp[1:] != sl_grp[:-1]
    first_slot_of_grp = np.zeros(NBLK * NSPLIT, np.int64)
    first_slot_of_grp[sl_grp[sl_first]] = sl_ids[sl_first]
    choff = np.zeros(NSPLIT, np.int64)
    choff[1:] = np.cumsum(np.asarray(CHS[:-1], np.int64) * 128)
    pos = (slot_glob - first_slot_of_grp[grp]) + choff[half]

    # few-hot tiles: oh[blk, pos%128, pos//128, rloc] = w  (per edge)
    ohg = np.zeros((NBLK, 128, CH, 128), dtype=G_NP)
    ohg[blk, pos % 128, pos >> 7, r_s & 127] = w_s.astype(G_NP)

    # gather indices (one per slot; row index within its half-table)
    pad_c = np.zeros((NBLK, slots), np.int16)
    pad_c[blk[newslot], pos[newslot]] = tabrow[newslot].astype(np.int16)
    wrap = pad_c.reshape(NBLK, CH * 8, 16).transpose(0, 2, 1)  # [NBLK,16,CH*8]
    wrap = np.tile(wrap, (1, 8, 1))                            # [NBLK,128,CH*8]

    xT = np.ascontiguousarray(x.T)                             # [128, N]

    ba_nonzero = bool(np.any(ba))
    bf_nonzero = bool(np.any(bf))

    in_maps = []
    for m in range(NCORES):
        im = {
            "xT": np.ascontiguousarray(xT[:, m * SHARD:(m + 1) * SHARD]),
            "wa": Wa,
            "wf": Wf,
            "oh": np.ascontiguousarray(
                ohg[m * BLKS:(m + 1) * BLKS].reshape(BLKS, 128, CH * 128)),
            "idxs": np.ascontiguousarray(
                wrap[m * BLKS:(m + 1) * BLKS].transpose(1, 0, 2)
                .reshape(128, BLKS * CH * 8)),
        }
        if ba_nonzero:
            im["ba_bc"] = np.broadcast_to(ba, (128, K)).copy()
        if bf_nonzero:
            im["bf_bc"] = np.broadcast_to(bf, (128, CIN)).copy()
        in_maps.append(im)
    return CHS, ba_nonzero, bf_nonzero, in_maps


def kernel(**inputs):
    from concourse.bass_utils import run_bass_kernel_spmd

    CH, ba_nz, bf_nz, in_maps = _prep(inputs)

    key = (tuple(CH), ba_nz, bf_nz)
    if key not in _cache:
        _cache[key] = _build(CH, ba_nz, bf_nz)
    nc = _cache[key]

    res = run_bass_kernel_spmd(nc, in_maps, core_ids=list(range(NCORES)))
    kernel.last_results = res

    S = np.concatenate([np.asarray(res.results[m]["s_out"])
                        for m in range(NCORES)], axis=0)
    P = np.zeros((K, K), np.float64)
    PX = np.zeros((K, CIN), np.float64)
    for m in range(NCORES):
        P += np.asarray(res.results[m]["p_out"], dtype=np.float64)
        PX += np.asarray(res.results[m]["px_out"], dtype=np.float64)
    P /= SCALE
    pooled_adj = (P + P.T).astype(np.float32)
    pooled_x = PX.astype(np.float32)
    return pooled_x, pooled_adj, S
